# revision 1
# baseline (speedup 1.0000x reference)
"""Additive-attention Bass kernel for Trainium2, data-parallel over batch on 8 cores.

Math per batch b:
    q = queries[b] @ W_q                      # (H,)
    kp[t, h] = sum_d keys[b, t, d] W_k[d, h]  # (Tk, H)
    feat = tanh(q + kp)                       # (Tk, H)
    s[t] = feat[t] . w_v                      # (Tk,)
    attn = softmax(s)                         # = exp(s) / sum exp(s)  (no max-sub
                                              #   needed: |s| <= ||w_v||_1 ~ 13)
    out[b] = attn @ values[b]                 # (H,)

On-chip layout: features are [h(partitions), t(free)] so ACT applies the q bias
per-partition and the w_v dot is a single M=1 matmul. keys arrive [t, d] and are
transposed to [d, t] in-PE (pass-through transpose matmuls). exp(s) rows are
moved to columns with K=1 transpose matmuls to feed the values matmul, which
accumulates one PSUM bank across the whole batch; normalization by 1/sum(exp)
happens once at the end.
"""

import numpy as np

import concourse.bass as bass
import concourse.mybir as mybir
import concourse.tile as tile
from concourse import bacc
from concourse.bass import ts
from concourse.bass_utils import run_bass_kernel_spmd
from concourse.masks import make_identity

B, TK, D, H = 32, 8192, 256, 256
NCORES = 8
BL = B // NCORES          # batches per core
CHUNK = 512               # t-chunk per compute iteration
NCHUNK = TK // CHUNK
NSUB = CHUNK // 128
TT = 4096                 # t-span per DMA load (32 KB contiguous per partition)
NL = TK // TT             # loads per batch
NCC = TT // CHUNK         # compute chunks per load
NNT = TT // 128           # n-slices per load tile

F32 = mybir.dt.float32
F32R = mybir.dt.float32r
F16 = mybir.dt.float16
AF = mybir.ActivationFunctionType


def build():
    nc = bacc.Bacc("TRN2", target_bir_lowering=False, debug=False, num_devices=NCORES)
    keys_d = nc.dram_tensor("keys", [BL, TK, D], F32, kind="ExternalInput").ap()
    vals_d = nc.dram_tensor("values", [BL, TK, D], F32R, kind="ExternalInput").ap()
    qrs_d = nc.dram_tensor("queries", [BL, D], F32, kind="ExternalInput").ap()
    wq_d = nc.dram_tensor("W_q", [D, H], F32, kind="ExternalInput").ap()
    wk_d = nc.dram_tensor("W_k", [D, H], F32, kind="ExternalInput").ap()
    wv_d = nc.dram_tensor("w_v", [1, H], F32, kind="ExternalInput").ap()
    out_d = nc.dram_tensor("out", [BL, D], F32, kind="ExternalOutput").ap()

    with tile.TileContext(nc) as tc:
        with (
            tc.tile_pool(name="consts", bufs=1) as consts,
            tc.tile_pool(name="kin", bufs=2) as kin,
            tc.tile_pool(name="vin", bufs=2) as vin,
            tc.tile_pool(name="mid", bufs=2) as mid,
            tc.tile_pool(name="small", bufs=2) as small,
        ):
            ident_f32 = consts.tile([128, 128], F32)
            make_identity(nc, ident_f32)
            ident = consts.tile([128, 128], F16)
            nc.vector.tensor_copy(out=ident, in_=ident_f32)
            one11 = consts.tile([1, 1], F32)
            nc.vector.memset(one11, 1.0)
            ones_col = consts.tile([128, 1], F32)
            nc.vector.memset(ones_col, 1.0)
            negc = consts.tile([128, 1], F32)
            nc.vector.memset(negc, -6.0)

            wk_f32 = consts.tile([128, 2, H], F32)
            nc.sync.dma_start(out=wk_f32, in_=wk_d.rearrange("(dt p) h -> p dt h", p=128))
            wk_s = consts.tile([128, 2, H], F16)
            nc.vector.tensor_copy(out=wk_s, in_=wk_f32)
            wq_s = consts.tile([128, 2, H], F32)
            nc.sync.dma_start(out=wq_s, in_=wq_d.rearrange("(dt p) h -> p dt h", p=128))
            wv_row = consts.tile([1, H], F32)
            nc.sync.dma_start(out=wv_row, in_=wv_d)
            q_rows = consts.tile([1, BL * D], F32)
            nc.sync.dma_start(
                out=q_rows, in_=qrs_d.rearrange("b d -> (b d)").rearrange("(o f) -> o f", o=1)
            )

            wv_cols = consts.tile([128, 2], F16)      # w_v as [h, htile] columns
            q_cols = consts.tile([128, BL, 2], F32)  # q biases [h, b, htile]

            # ---- setup: w_v columns and per-batch q biases (all tiny) ----
            with tc.tile_pool(name="setup_ps", bufs=1, space="PSUM") as setup_ps:
                ps_wv = setup_ps.tile([128, 2], F32)
                for ht in range(2):
                    nc.tensor.matmul(
                        out=ps_wv[:, ht : ht + 1],
                        lhsT=wv_row[0:1, ts(ht, 128)],
                        rhs=one11,
                        is_transpose=True,
                    )
                nc.vector.tensor_copy(out=wv_cols, in_=ps_wv)

                for b in range(BL):
                    ps_qc = setup_ps.tile([128, 2], F32, tag="ps_qc")
                    for dt in range(2):
                        nc.tensor.matmul(
                            out=ps_qc[:, dt : dt + 1],
                            lhsT=q_rows[0:1, b * D + dt * 128 : b * D + (dt + 1) * 128],
                            rhs=one11,
                            is_transpose=True,
                        )
                    qc_s = small.tile([128, 2], F32, tag="qc_s")
                    nc.vector.tensor_copy(out=qc_s, in_=ps_qc)
                    ps_q = setup_ps.tile([128, 2], F32, tag="ps_q")
                    for ht in range(2):
                        for dt in range(2):
                            nc.tensor.matmul(
                                out=ps_q[:, ht : ht + 1],
                                lhsT=wq_s[:, dt, ts(ht, 128)],
                                rhs=qc_s[:, dt : dt + 1],
                                start=(dt == 0),
                                stop=(dt == 1),
                            )
                    nc.vector.tensor_copy(out=q_cols[:, b, :], in_=ps_q)

            # ---- main loop ----
            with (
                tc.tile_pool(name="ptr", bufs=1, space="PSUM") as ptrp,
                tc.tile_pool(name="pkp", bufs=3, space="PSUM") as pkpp,
                tc.tile_pool(name="scol", bufs=2, space="PSUM") as scolp,
                tc.tile_pool(name="pout", bufs=2, space="PSUM") as poutp,
            ):
                for b in range(BL):
                    psum_out = poutp.tile([1, D], F32, tag="psum_out")
                    z_pp = small.tile([128, NCHUNK], F32, tag="z_pp")

                    # lag-1 pipeline state: (esc, vals_nat, c) awaiting ec+values
                    pend = []

                    def flush_pend(last):
                        ec_p, vals_p, c_p = pend.pop(0)
                        cc_p = c_p % NCC
                        for j in range(NSUB):
                            nc.tensor.matmul(
                                out=psum_out,
                                lhsT=ec_p[:, j : j + 1],
                                rhs=vals_p[:, cc_p * NSUB + j, :],
                                start=(c_p == 0 and j == 0),
                                stop=(last and j == NSUB - 1),
                                skip_group_check=True,
                            )

                    for L in range(NL):
                        # p-major split: partition p holds t = L*TT + p*NNT + nn,
                        # giving a fully contiguous 16 KB per partition per DMA.
                        # The resulting t-permutation is applied consistently to
                        # keys and values, and softmax is order-invariant.
                        keys_f32 = kin.tile([128, NNT, D], F32, tag="keys_f32")
                        nc.sync.dma_start(
                            out=keys_f32,
                            in_=keys_d[b, L * TT : (L + 1) * TT, :].rearrange(
                                "(p n) d -> p n d", p=128
                            ),
                        )
                        vals_all = vin.tile([128, NNT, D], F32R, tag="vals_all")
                        nc.sync.dma_start(
                            out=vals_all,
                            in_=vals_d[b, L * TT : (L + 1) * TT, :].rearrange(
                                "(p n) d -> p n d", p=128
                            ),
                        )
                        for cc in range(NCC):
                            c = L * NCC + cc
                            keys_nat = kin.tile([128, NSUB, D], F16, tag="keys16", bufs=4)
                            nc.vector.tensor_copy(
                                out=keys_nat,
                                in_=keys_f32[:, cc * NSUB : (cc + 1) * NSUB, :],
                            )

                            # keys [t, d] -> [d, t] via PE pass-through transposes
                            ptr_t = ptrp.tile([128, 2, NSUB, 128], F16, tag="ptr_t")
                            for j in range(NSUB):
                                for dt in range(2):
                                    nc.tensor.matmul(
                                        out=ptr_t[:, dt, j, :],
                                        lhsT=keys_nat[:, j, ts(dt, 128)],
                                        rhs=ident,
                                        is_transpose=True,
                                    )
                            kT = mid.tile([128, 2, CHUNK], F16, tag="kT")
                            for dt in range(2):
                                nc.vector.tensor_copy(
                                    out=kT[:, dt, :], in_=ptr_t[:, dt, :, :]
                                )

                            # kp[h, t] = W_k^T keys^T
                            kps = []
                            for ht in range(2):
                                kp = pkpp.tile([128, CHUNK], F32, tag="kp")
                                kps.append(kp)
                                for dt in range(2):
                                    nc.tensor.matmul(
                                        out=kp,
                                        lhsT=wk_s[:, dt, ts(ht, 128)],
                                        rhs=kT[:, dt, :],
                                        start=(dt == 0),
                                        stop=(dt == 1),
                                    )

                            # feat = tanh(kp + q)   (q as per-partition bias)
                            feat = mid.tile([128, 2, CHUNK], F16, tag="feat")
                            for ht in range(2):
                                nc.scalar.activation(
                                    out=feat[:, ht, :],
                                    in_=kps[ht],
                                    func=AF.Tanh,
                                    bias=q_cols[:, b, ht : ht + 1],
                                    scale=1.0,
                                )

                            # scores directly as columns: feat tiles stationary,
                            # w_v as the N=1 moving operand
                            scol = scolp.tile([128, NSUB], F32, tag="scol")
                            for j in range(NSUB):
                                for ht in range(2):
                                    nc.tensor.matmul(
                                        out=scol[:, j : j + 1],
                                        lhsT=feat[:, ht, ts(j, 128)],
                                        rhs=wv_cols[:, ht : ht + 1],
                                        start=(ht == 0),
                                        stop=(ht == 1),
                                    )

                            # e-columns = exp(scol - 6); per-partition Z partials
                            ec = small.tile([128, NSUB], F32R, tag="ec")
                            nc.scalar.activation(
                                out=ec,
                                in_=scol,
                                func=AF.Exp,
                                bias=negc[:, 0:1],
                                accum_out=z_pp[:, c : c + 1],
                            )

                            pend.append((ec, vals_all, c))
                            if len(pend) > 1:
                                flush_pend(last=False)

                    flush_pend(last=True)

                    # normalize: out = out_raw / Z
                    # Z = sum over partitions and chunks of z_pp:
                    #   [128,16] x ones -> [16,1] -> transpose -> [1,16] -> reduce
                    zt_ps = scolp.tile([16, 1], F32, tag="scol", name="zt_ps")
                    nc.tensor.matmul(out=zt_ps, lhsT=z_pp, rhs=ones_col)
                    zt_s = small.tile([16, 1], F32, tag="zt_s")
                    nc.vector.tensor_copy(out=zt_s, in_=zt_ps)
                    zrow_ps = scolp.tile([1, 16], F32, tag="scol", name="zrow_ps")
                    nc.tensor.matmul(
                        out=zrow_ps,
                        lhsT=zt_s,
                        rhs=ident_f32[0:16, 0:16],
                        is_transpose=True,
                    )
                    z = small.tile([1, 1], F32, tag="z")
                    nc.vector.reduce_sum(out=z, in_=zrow_ps, axis=mybir.AxisListType.X)
                    rz = small.tile([1, 1], F32, tag="rz")
                    nc.vector.reciprocal(out=rz, in_=z)
                    orow = small.tile([1, D], F32, tag="orow")
                    nc.scalar.mul(out=orow, in_=psum_out, mul=rz[0:1, 0:1])
                    nc.sync.dma_start(out=out_d[b : b + 1, :], in_=orow)

    nc.compile()
    return nc


_NC = None


def _get_nc():
    global _NC
    if _NC is None:
        _NC = build()
    return _NC


def kernel(queries, keys, values, W_q, W_k, w_v):
    nc = _get_nc()
    queries = np.asarray(queries, np.float32)
    keys = np.asarray(keys, np.float32)
    values = np.asarray(values, np.float32)
    W_q = np.ascontiguousarray(np.asarray(W_q, np.float32))
    W_k = np.ascontiguousarray(np.asarray(W_k, np.float32))
    wv2 = np.ascontiguousarray(np.asarray(w_v, np.float32).reshape(1, H))
    in_maps = []
    for i in range(NCORES):
        sl = slice(i * BL, (i + 1) * BL)
        in_maps.append(
            {
                "queries": np.ascontiguousarray(queries[sl]),
                "keys": np.ascontiguousarray(keys[sl]),
                "values": np.ascontiguousarray(values[sl]),
                "W_q": W_q,
                "W_k": W_k,
                "w_v": wv2,
            }
        )
    res = run_bass_kernel_spmd(nc, in_maps, list(range(NCORES)))
    return np.concatenate([res.results[i]["out"] for i in range(NCORES)], axis=0)

